# revision 1
# baseline (speedup 1.0000x reference)
"""ConvTransformerEncoderLayer on 8 trn2 NeuronCores.

Sharding: pure data-parallel over batch (B=8 -> 1 batch element per core).
Each core runs the full layer for its batch element; no collectives.

Per-core layout strategy (S=1024, D=512, H=8, hd=64, DFF=2048):
  - Q,K convs produce [c, s] (channel-on-partition) via fp32r matmuls.
  - V conv produces V^T [t, c] (+ a ones column per head) so the AV matmul
    emits av^T [d, s] directly with the softmax denominator as a spare row.
  - softmax without max-subtraction (scores are O(10), fp32 exp is safe).
  - av^T feeds Wo directly (no transpose); x -> x^T via PE transpose for FFN.
  - FFN runs in two sequence-half passes to halve hidden-state SBUF.
  - bv and bo are folded into one bias row on the host (softmax rows sum to 1).
All matmuls use float32r (TF32-like, full PE rate); everything else fp32.
"""
import sys

sys.path.insert(0, "/opt/trn_rl_repo")
import numpy as np

P = 128          # partitions
S = 1024         # sequence
D = 512          # d_model
H = 8            # heads
HD = 64          # head dim
DFF = 2048
KS = 3           # conv kernel size
EPS = 1e-5
NCORES = 8
CT = D // P      # 4 channel tiles
ST = S // P      # 8 sequence tiles
FT = DFF // P    # 16 ff tiles
SH = 512         # matmul free-dim chunk (= psum bank)

_CACHE = {}


def _build_nc():
    import concourse.tile as tile
    from concourse import bacc, mybir

    f32 = mybir.dt.float32
    f32r = mybir.dt.float32r
    AF = mybir.ActivationFunctionType
    ALU = mybir.AluOpType

    nc = bacc.Bacc("TRN2", target_bir_lowering=False, debug=False,
                   enable_asserts=False, num_devices=NCORES)

    def din(name, shape, dt=f32r):
        return nc.dram_tensor(name, shape, dt, kind="ExternalInput").ap()

    srcT = din("srcT", [P, CT, S])             # src^T tiled: [p, ct, s]
    src_sd = din("src_sd", [P, ST, D], f32)    # src tiled:   [p, st, d]
    wq_d = din("wq", [P, CT, KS, D])           # Wq[co, ci, k] -> [ci_p, ci_t, k, co]
    wk_d = din("wk", [P, CT, KS, D])
    wv_d = din("wv", [P, CT, D])
    wo_d = din("wo", [P, CT, D])               # Wo[e, d] -> [d_p, d_t, e]
    w1_d = din("w1", [P, CT, DFF])             # W1[f, d] -> [d_p, d_t, f]
    w2_d = din("w2", [P, FT, D])               # W2[e, f] -> [f_p, f_t, e]
    bq_d = din("bq", [P, CT], f32)
    bk_d = din("bk", [P, CT], f32)
    b1_d = din("b1", [P, FT], f32)
    bo2_d = din("bo2", [1, D])                 # bo + Wo @ bv
    b2_d = din("b2r", [1, D])
    g1_d = din("g1r", [P, D], f32)
    be1_d = din("be1r", [P, D], f32)
    g2_d = din("g2r", [P, D], f32)
    be2_d = din("be2r", [P, D], f32)
    id_d = din("ident", [P, P], f32)
    ones_d = din("onesrow", [1, P])

    out_d = nc.dram_tensor("out", [P, ST, D], f32, kind="ExternalOutput").ap()
    import os
    dbg = os.environ.get("KERNEL_DEBUG_TAPS", "0") == "1"
    phase = int(os.environ.get("KERNEL_PHASE", "4"))
    if dbg:
        dbg_d = {
            "dQ": nc.dram_tensor("dQ", [P, CT, S], f32r, kind="ExternalOutput").ap(),
            "dK": nc.dram_tensor("dK", [P, CT, S], f32r, kind="ExternalOutput").ap(),
            "dV": nc.dram_tensor("dV", [P, ST, H, HD + 1], f32r, kind="ExternalOutput").ap(),
            "dET": nc.dram_tensor("dET", [P, ST, SH], f32r, kind="ExternalOutput").ap(),
            "dAVT": nc.dram_tensor("dAVT", [P, CT, S], f32r, kind="ExternalOutput").ap(),
            "dXS": nc.dram_tensor("dXS", [P, ST, D], f32, kind="ExternalOutput").ap(),
            "dXT": nc.dram_tensor("dXT", [P, CT, S], f32r, kind="ExternalOutput").ap(),
        }

    class _PhaseDone(Exception):
        pass

    with tile.TileContext(nc) as tc:
      try:
        with (
            tc.tile_pool(name="big", bufs=1) as big,
            tc.tile_pool(name="etp", bufs=2) as etp,
            tc.tile_pool(name="small", bufs=1) as small,
            tc.tile_pool(name="tmp", bufs=3) as tmp,
            tc.tile_pool(name="tiny", bufs=4) as tiny,
            tc.tile_pool(name="nrm", bufs=1) as nrm,
            tc.tile_pool(name="psp", bufs=8, space="PSUM") as psp,
        ):
            # ---------- static small constants ----------
            identity = small.tile([P, P], f32, tag="ident")
            nc.sync.dma_start(identity[:], id_d[:])
            onesrow = small.tile([1, P], f32r, tag="ones")
            nc.sync.dma_start(onesrow[:], ones_d[:])
            bq_t = small.tile([P, CT], f32, tag="bq")
            nc.sync.dma_start(bq_t[:], bq_d[:])
            bk_t = small.tile([P, CT], f32, tag="bk")
            nc.sync.dma_start(bk_t[:], bk_d[:])
            b1_t = small.tile([P, FT], f32, tag="b1")
            nc.sync.dma_start(b1_t[:], b1_d[:])
            bo2_t = small.tile([1, D], f32r, tag="bo2")
            nc.sync.dma_start(bo2_t[:], bo2_d[:])
            b2_t = small.tile([1, D], f32r, tag="b2")
            nc.sync.dma_start(b2_t[:], b2_d[:])
            g1_t = small.tile([P, D], f32, tag="g1")
            nc.sync.dma_start(g1_t[:], g1_d[:])
            be1_t = small.tile([P, D], f32, tag="be1")
            nc.sync.dma_start(be1_t[:], be1_d[:])
            g2_t = small.tile([P, D], f32, tag="g2")
            nc.sync.dma_start(g2_t[:], g2_d[:])
            be2_t = small.tile([P, D], f32, tag="be2")
            nc.sync.dma_start(be2_t[:], be2_d[:])
            epsv = small.tile([P, 1], f32, tag="eps")
            nc.vector.memset(epsv[:], EPS)

            # ---------- inputs / conv weights ----------
            X = big.tile([P, CT, S + 2], f32r, tag="A", name="X")
            nc.vector.memset(X[:, :, 0:1].bitcast(f32), 0.0)
            nc.vector.memset(X[:, :, S + 1:S + 2].bitcast(f32), 0.0)
            for ct in range(CT):
                eng = nc.sync if ct % 2 == 0 else nc.scalar
                eng.dma_start(X[:, ct, 1:S + 1], srcT[:, ct, :])
            wq = big.tile([P, CT, KS, D], f32r, tag="WA", name="wq_s")
            nc.sync.dma_start(wq[:], wq_d[:])
            wk = big.tile([P, CT, KS, D], f32r, tag="WB", name="wk_s")
            nc.scalar.dma_start(wk[:], wk_d[:])
            wv = big.tile([P, CT, D], f32r, tag="WC", name="wv_s")
            nc.scalar.dma_start(wv[:], wv_d[:])

            Q = big.tile([P, CT, S], f32r, tag="Q", name="Q")
            K = big.tile([P, CT, S], f32r, tag="K", name="K")
            VTx = big.tile([P, ST, H, HD + 1], f32r, tag="V", name="VTx")
            AVT = big.tile([P, CT, S], f32r, tag="AVT", name="AVT")

            # ---------- V conv -> VTx (V^T with a ones column per head) ----------
            nc.vector.memset(VTx[:, :, :, HD:HD + 1].bitcast(f32), 1.0)
            for tt in range(ST):
                ps = psp.tile([P, SH], f32, tag="ps", bufs=6, name="psv")
                for ci in range(CT):
                    nc.tensor.matmul(ps[:], X[:, ci, 1 + tt * P:1 + (tt + 1) * P],
                                     wv[:, ci, :],
                                     start=(ci == 0), stop=(ci == CT - 1))
                nc.vector.tensor_copy(VTx[:, tt, :, 0:HD],
                                      ps.rearrange("p (h e) -> p h e", h=H))

            # ---------- Q/K convs (per output channel tile) + attention ----------
            def conv_qk(dst, w, bias_t, ct):
                for sc in range(2):
                    ps = psp.tile([P, SH], f32, tag="ps", bufs=6, name="psqk")
                    first = True
                    for ci in range(CT):
                        for k in range(KS):
                            nc.tensor.matmul(
                                ps[:], w[:, ci, k, ct * P:(ct + 1) * P],
                                X[:, ci, sc * SH + k: sc * SH + k + SH],
                                start=first, stop=(ci == CT - 1 and k == KS - 1))
                            first = False
                    nc.vector.tensor_scalar_add(dst[:, ct, sc * SH:(sc + 1) * SH],
                                                ps[:], bias_t[:, ct:ct + 1])

            def attention_head(h):
                base = HD * (h % 2)
                qh = Q[base:base + HD, h // 2, :]
                kh = K[base:base + HD, h // 2, :]
                for sc in range(2):
                    et = etp.tile([P, ST, SH], f32r, tag="ET", name=f"et{h}_{sc}")
                    for tt in range(ST):
                        ps = psp.tile([P, SH], f32, tag="ps", bufs=6, name="pssc")
                        nc.tensor.matmul(ps[:], kh[:, tt * P:(tt + 1) * P],
                                         qh[:, sc * SH:(sc + 1) * SH],
                                         start=True, stop=True)
                        nc.scalar.activation(et[:, tt, :], ps[:], AF.Exp,
                                             bias=0.0, scale=1.0 / HD)
                    if dbg and h == 0 and sc == 0:
                        nc.sync.dma_start(dbg_d["dET"][:], et[:])
                    # AV: av^T[d, s] accumulated over t tiles; row HD = denom
                    avps = psp.tile([P, SH], f32, tag="ps", bufs=6, name="avps")
                    for tt in range(ST):
                        nc.tensor.matmul(avps[0:HD + 1, :], VTx[:, tt, h, :],
                                         et[:, tt, :],
                                         start=(tt == 0), stop=(tt == ST - 1))
                    # softmax normalization: recip of denom row, broadcast, mult
                    rrec = nrm.tile([1, SH], f32r, tag="rrec", name="rrec")
                    with nc.allow_low_precision(reason="f32r softmax denom"):
                        nc.vector.reciprocal(rrec[0:1, :], avps[HD:HD + 1, :])
                    rrep = nrm.tile([HD, SH], f32r, tag="rrep", name="rrep")
                    nc.gpsimd.partition_broadcast(rrep[:], rrec[0:1, :])
                    base_o = HD * (h % 2)
                    nc.vector.tensor_tensor(
                        out=AVT[base_o:base_o + HD, h // 2, sc * SH:(sc + 1) * SH],
                        in0=avps[0:HD, :], in1=rrep[:], op=ALU.mult)

            for ct in range(CT):
                conv_qk(Q, wq, bq_t, ct)
                conv_qk(K, wk, bk_t, ct)
                if phase >= 2:
                    attention_head(2 * ct)
                    attention_head(2 * ct + 1)
            if phase < 2:
                nc.sync.dma_start(out_d.bitcast(f32r)[:], Q[:])
                raise _PhaseDone()

            if dbg:
                nc.sync.dma_start(dbg_d["dQ"][:], Q[:])
                nc.sync.dma_start(dbg_d["dK"][:], K[:])
                nc.sync.dma_start(dbg_d["dV"][:], VTx[:])
                nc.sync.dma_start(dbg_d["dAVT"][:], AVT[:])

            if phase < 3:
                nc.sync.dma_start(out_d.bitcast(f32r)[:], AVT[:])
                raise _PhaseDone()

            # FFN weights / later tensors into slots freed by earlier phases
            wo = big.tile([P, CT, D], f32r, tag="WC", name="wo_s")
            nc.sync.dma_start(wo[:], wo_d[:])
            w1 = big.tile([P, CT, DFF], f32r, tag="WA", name="w1_s")
            nc.scalar.dma_start(w1[:], w1_d[:])
            w2 = big.tile([P, FT, D], f32r, tag="WB", name="w2_s")
            nc.sync.dma_start(w2[:], w2_d[:])
            srcs = big.tile([P, ST, D], f32, tag="V", name="srcs")
            nc.scalar.dma_start(srcs[:], src_sd[:])

            xs = big.tile([P, ST, D], f32, tag="K", name="xs")  # LN1 out [s, d]
            xT = big.tile([P, CT, S], f32r, tag="Q", name="xT")
            y = big.tile([P, ST, D], f32, tag="A", name="y")

            def layernorm(dst, z, g_t, be_t):  # uses epsv from enclosing scope
                stats = tiny.tile([P, 6], f32, tag="st6", name="st6")
                nc.vector.bn_stats(stats[:], z[:])
                mv = tiny.tile([P, 2], f32, tag="mv", name="mv")
                nc.vector.bn_aggr(mv[:], stats[:])
                sd = tiny.tile([P, 1], f32, tag="sd", name="sd")
                nc.scalar.activation(sd[:], mv[:, 1:2], AF.Sqrt, bias=epsv[:], scale=1.0)
                rstd = tiny.tile([P, 1], f32, tag="rstd", name="rstd")
                nc.vector.reciprocal(rstd[:], sd[:])
                xn = tmp.tile([P, D], f32, tag="t2", bufs=1, name="xn")
                nc.vector.tensor_scalar(out=xn[:], in0=z[:], scalar1=mv[:, 0:1],
                                        scalar2=rstd[:], op0=ALU.subtract,
                                        op1=ALU.mult)
                nc.vector.tensor_tensor(out=xn[:], in0=xn[:], in1=g_t[:],
                                        op=ALU.mult)
                nc.vector.tensor_tensor(out=dst, in0=xn[:], in1=be_t[:],
                                        op=ALU.add)

            # ---------- Wo projection + residual + LN1 ----------
            for st in range(ST):
                ps = psp.tile([P, SH], f32, tag="ps", bufs=6, name="pswo")
                for dt in range(CT):
                    nc.tensor.matmul(ps[:], AVT[:, dt, st * P:(st + 1) * P],
                                     wo[:, dt, :], start=(dt == 0), stop=False)
                nc.tensor.matmul(ps[:], onesrow[:], bo2_t[:],
                                 start=False, stop=True)
                z = tmp.tile([P, D], f32, tag="t1", bufs=1, name="z1")
                nc.vector.tensor_tensor(out=z[:], in0=ps[:], in1=srcs[:, st, :],
                                        op=ALU.add)
                layernorm(xs[:, st, :], z, g1_t, be1_t)

            # ---------- x -> x^T for FFN ----------
            for dt in range(CT):
                for st in range(ST):
                    tp = psp.tile([P, P], f32, tag="tp", bufs=2, name="tp")
                    nc.tensor.transpose(tp[:], xs[:, st, dt * P:(dt + 1) * P],
                                        identity[:])
                    nc.vector.tensor_copy(xT[:, dt, st * P:(st + 1) * P], tp[:])

            if dbg:
                nc.sync.dma_start(dbg_d["dXS"][:], xs[:])
                nc.sync.dma_start(dbg_d["dXT"][:], xT[:])
            if phase < 4:
                nc.sync.dma_start(out_d[:], xs[:])
                raise _PhaseDone()

            # ---------- FFN in two sequence-half passes ----------
            for sc in range(2):
                hT = [etp.tile([P, FT // 2, SH], f32r, tag="ET",
                               name=f"hT{sc}_{i}") for i in range(2)]
                for ft in range(FT):
                    ps = psp.tile([P, SH], f32, tag="ps", bufs=6, name="psf1")
                    for dt in range(CT):
                        nc.tensor.matmul(ps[:], w1[:, dt, ft * P:(ft + 1) * P],
                                         xT[:, dt, sc * SH:(sc + 1) * SH],
                                         start=(dt == 0), stop=(dt == CT - 1))
                    nc.scalar.activation(hT[ft // 8][:, ft % 8, :], ps[:], AF.Relu,
                                         bias=b1_t[:, ft:ft + 1], scale=1.0)
                for j in range(ST // 2):
                    st = sc * (ST // 2) + j
                    ps = psp.tile([P, SH], f32, tag="ps", bufs=6, name="psf2")
                    for ft in range(FT):
                        nc.tensor.matmul(
                            ps[:], hT[ft // 8][:, ft % 8, j * P:(j + 1) * P],
                            w2[:, ft, :], start=(ft == 0), stop=False)
                    nc.tensor.matmul(ps[:], onesrow[:], b2_t[:],
                                     start=False, stop=True)
                    z = tmp.tile([P, D], f32, tag="t1", bufs=1, name="z2")
                    nc.vector.tensor_tensor(out=z[:], in0=ps[:], in1=xs[:, st, :],
                                            op=ALU.add)
                    layernorm(y[:, st, :], z, g2_t, be2_t)
                    nc.sync.dma_start(out_d[:, st, :], y[:, st, :])

      except _PhaseDone:
        pass
    nc.compile()
    return nc


def _prep_inputs(src, Wq, bq, Wk, bk, Wv, bv, Wo, bo, W1, b1, W2, b2,
                 g1, be1, g2, be2):
    f = np.float32

    def ctile(w):  # [co, ci(, k)] conv weight -> [p, ci_t(, k), co]
        wt = np.ascontiguousarray(np.moveaxis(w, 0, -1))  # [ci(,k), co]
        return np.ascontiguousarray(
            wt.reshape(CT, P, *wt.shape[1:]).transpose(1, 0, *range(2, wt.ndim + 1)))

    shared = {
        "wq": ctile(Wq).astype(f),                       # [P, CT, KS, D]
        "wk": ctile(Wk).astype(f),
        "wv": ctile(Wv[:, :, 0]).astype(f),              # [P, CT, D]
        "wo": np.ascontiguousarray(
            Wo.T.reshape(CT, P, D).transpose(1, 0, 2)).astype(f),
        "w1": np.ascontiguousarray(
            W1.T.reshape(CT, P, DFF).transpose(1, 0, 2)).astype(f),
        "w2": np.ascontiguousarray(
            W2.T.reshape(FT, P, D).transpose(1, 0, 2)).astype(f),
        "bq": np.ascontiguousarray(bq.reshape(CT, P).T).astype(f),
        "bk": np.ascontiguousarray(bk.reshape(CT, P).T).astype(f),
        "b1": np.ascontiguousarray(b1.reshape(FT, P).T).astype(f),
        "bo2": (bo + Wo @ bv).reshape(1, D).astype(f),
        "b2r": b2.reshape(1, D).astype(f),
        "g1r": np.ascontiguousarray(np.broadcast_to(g1, (P, D))).astype(f),
        "be1r": np.ascontiguousarray(np.broadcast_to(be1, (P, D))).astype(f),
        "g2r": np.ascontiguousarray(np.broadcast_to(g2, (P, D))).astype(f),
        "be2r": np.ascontiguousarray(np.broadcast_to(be2, (P, D))).astype(f),
        "ident": np.eye(P, dtype=f),
        "onesrow": np.ones((1, P), dtype=f),
    }
    in_maps = []
    for b in range(NCORES):
        m = dict(shared)
        m["srcT"] = np.ascontiguousarray(
            src[b].T.reshape(CT, P, S).transpose(1, 0, 2)).astype(f)
        m["src_sd"] = np.ascontiguousarray(
            src[b].reshape(ST, P, D).transpose(1, 0, 2)).astype(f)
        in_maps.append(m)
    return in_maps


def get_nc():
    if "nc" not in _CACHE:
        _CACHE["nc"] = _build_nc()
    return _CACHE["nc"]


def kernel(**inputs):
    from concourse.bass_utils import run_bass_kernel_spmd

    in_maps = _prep_inputs(**{k: np.asarray(v) for k, v in inputs.items()})
    nc = get_nc()
    res = run_bass_kernel_spmd(nc, in_maps, core_ids=list(range(NCORES)))
    outs = [r["out"].transpose(1, 0, 2).reshape(S, D) for r in res.results]
    return np.stack(outs).astype(np.float32)



# revision 24
# speedup vs baseline: 1.3504x; 1.3504x over previous
"""ConvTransformerEncoderLayer on 8 trn2 NeuronCores.

Sharding: pure data-parallel over batch (B=8 -> 1 batch element per core).
Each core runs the full layer for its batch element; no collectives.

Per-core layout strategy (S=1024, D=512, H=8, hd=64, DFF=2048):
  - Q,K convs produce [c, s] (channel-on-partition) via fp32r matmuls.
  - V conv produces V^T [t, c] (+ a ones column per head) so the AV matmul
    emits av^T [d, s] directly with the softmax denominator as a spare row.
  - softmax without max-subtraction (scores are O(10), fp32 exp is safe).
  - av^T feeds Wo directly (no transpose); x -> x^T via PE transpose for FFN.
  - FFN runs in two sequence-half passes to halve hidden-state SBUF.
  - bv and bo are folded into one bias row on the host (softmax rows sum to 1).
All matmuls use float32r (TF32-like, full PE rate); everything else fp32.
"""
import sys

sys.path.insert(0, "/opt/trn_rl_repo")
import numpy as np

P = 128          # partitions
S = 1024         # sequence
D = 512          # d_model
H = 8            # heads
HD = 64          # head dim
DFF = 2048
KS = 3           # conv kernel size
EPS = 1e-5
NCORES = 8
CT = D // P      # 4 channel tiles
ST = S // P      # 8 sequence tiles
FT = DFF // P    # 16 ff tiles
SH = 512         # matmul free-dim chunk (= psum bank)

_CACHE = {}


def _build_nc():
    import concourse.tile as tile
    from concourse import bacc, mybir

    f32 = mybir.dt.float32
    f32r = mybir.dt.float32r
    AF = mybir.ActivationFunctionType
    ALU = mybir.AluOpType

    nc = bacc.Bacc("TRN2", target_bir_lowering=False, debug=False,
                   enable_asserts=False, num_devices=NCORES)

    def din(name, shape, dt=f32r):
        return nc.dram_tensor(name, shape, dt, kind="ExternalInput").ap()

    bf16 = mybir.dt.bfloat16
    srcT = din("srcT", [P, CT, S], f32)        # src^T tiled: [p, ct, s]
    wq_d = din("wq", [P, CT, KS, D], bf16)     # Wq[co, ci, k] -> [ci_p, ci_t, k, co]
    wk_d = din("wk", [P, CT, KS, D], bf16)
    wv_d = din("wv", [P, CT, D], bf16)
    wo_d = din("wo", [P, CT, D], bf16)         # Wo[e, d] -> [d_p, d_t, e]
    w1_d = din("w1", [P, CT, DFF], bf16)       # W1[f, d] -> [d_p, d_t, f]
    w2_d = din("w2", [P, FT, D], bf16)         # W2[e, f] -> [f_p, f_t, e]
    bq_d = din("bq", [P, CT], f32)
    bk_d = din("bk", [P, CT], f32)
    b1_d = din("b1", [P, FT], f32)
    bo2_d = din("bo2", [1, D], bf16)           # bo + Wo @ bv
    b2_d = din("b2r", [1, D], bf16)
    g1_d = din("g1r", [1, D], f32)
    be1_d = din("be1r", [1, D], f32)
    g2_d = din("g2r", [1, D], f32)
    be2_d = din("be2r", [1, D], f32)
    id_d = din("ident", [P, P], f32)
    ones_d = din("onesrow", [1, P], bf16)

    out_d = nc.dram_tensor("out", [P, ST, D], f32, kind="ExternalOutput").ap()
    import os
    dbg = os.environ.get("KERNEL_DEBUG_TAPS", "0") == "1"
    if dbg:
        dbg_d = {
            "dQ": nc.dram_tensor("dQ", [P, CT, S], bf16, kind="ExternalOutput").ap(),
            "dK": nc.dram_tensor("dK", [P, CT, S], bf16, kind="ExternalOutput").ap(),
            "dV": nc.dram_tensor("dV", [P, ST, H, HD + 1], bf16, kind="ExternalOutput").ap(),
            "dET": nc.dram_tensor("dET", [P, ST, SH], bf16, kind="ExternalOutput").ap(),
            "dAVT": nc.dram_tensor("dAVT", [P, CT, S], bf16, kind="ExternalOutput").ap(),
            "dXS": nc.dram_tensor("dXS", [P, ST, D], f32, kind="ExternalOutput").ap(),
            "dXT": nc.dram_tensor("dXT", [P, CT, S], bf16, kind="ExternalOutput").ap(),
        }

    class _PhaseDone(Exception):
        pass

    with tile.TileContext(nc) as tc:
      try:
        with (
            tc.tile_pool(name="big", bufs=1) as big,
            tc.tile_pool(name="etp", bufs=2) as etp,
            tc.tile_pool(name="small", bufs=1) as small,
            tc.tile_pool(name="tmp", bufs=3) as tmp,
            tc.tile_pool(name="tiny", bufs=4) as tiny,
            tc.tile_pool(name="nrm", bufs=1) as nrm,
            tc.tile_pool(name="psp", bufs=8, space="PSUM") as psp,
        ):
            # ---------- static small constants ----------
            identity = small.tile([P, P], f32, tag="ident")
            nc.sync.dma_start(identity[:], id_d[:])
            onesrow = small.tile([1, P], bf16, tag="ones")
            nc.sync.dma_start(onesrow[:], ones_d[:])
            bq_t = small.tile([P, CT], f32, tag="bq")
            nc.sync.dma_start(bq_t[:], bq_d[:])
            bk_t = small.tile([P, CT], f32, tag="bk")
            nc.sync.dma_start(bk_t[:], bk_d[:])
            b1_t = small.tile([P, FT], f32, tag="b1")
            nc.sync.dma_start(b1_t[:], b1_d[:])
            bo2_t = small.tile([1, D], bf16, tag="bo2")
            nc.sync.dma_start(bo2_t[:], bo2_d[:])
            b2_t = small.tile([1, D], bf16, tag="b2")
            nc.sync.dma_start(b2_t[:], b2_d[:])
            def bcast_row(nm, d):
                row = small.tile([1, D], f32, tag=nm + "row")
                nc.sync.dma_start(row[:], d[:])
                full = small.tile([P, D], f32, tag=nm)
                nc.gpsimd.partition_broadcast(full[:], row[0:1, :])
                return full

            g1_t = bcast_row("g1", g1_d)
            be1_t = bcast_row("be1", be1_d)
            g2_t = bcast_row("g2", g2_d)
            be2_t = bcast_row("be2", be2_d)
            epsv = small.tile([P, 1], f32, tag="eps")
            nc.vector.memset(epsv[:], EPS)

            # ---------- inputs / conv weights ----------
            X = big.tile([P, CT, S + 2], f32, tag="A", name="X")
            nc.vector.memset(X[:, :, 0:1], 0.0)
            nc.vector.memset(X[:, :, S + 1:S + 2], 0.0)
            for ct in range(CT):
                eng = nc.sync if ct % 2 == 0 else nc.scalar
                eng.dma_start(X[:, ct, 1:S + 1], srcT[:, ct, :])
            wq = big.tile([P, CT, KS, D], bf16, tag="WA", name="wq_s")
            nc.sync.dma_start(wq[:], wq_d[:])
            wk = big.tile([P, CT, KS, D], bf16, tag="WB", name="wk_s")
            nc.scalar.dma_start(wk[:], wk_d[:])
            wv = big.tile([P, CT, D], bf16, tag="WC", name="wv_s")
            nc.scalar.dma_start(wv[:], wv_d[:])

            # bf16 copy of X for matmuls (compiler requires both matmul
            # operands 16-bit when weights are bf16); f32 X kept for residual
            Xb = big.tile([P, CT, S + 2], bf16, tag="XB", name="Xb")
            for ci in range(CT):
                nc.vector.tensor_copy(Xb[:, ci, :], X[:, ci, :])

            Q = big.tile([P, CT, S], bf16, tag="Q", name="Q")
            K = big.tile([P, CT, S], bf16, tag="K", name="K")
            VTx = big.tile([P, ST, H, HD + 1], bf16, tag="V", name="VTx")
            AVT = big.tile([P, CT, S], bf16, tag="AVT", name="AVT")

            # ---------- V conv -> VTx (V^T with a ones column per head) ----------
            nc.vector.memset(VTx[:, :, :, HD:HD + 1], 1.0)
            for tt in range(ST):
                ps = psp.tile([P, SH], f32, tag="ps", bufs=6, name="psv")
                for ci in range(CT):
                    nc.tensor.matmul(ps[:], Xb[:, ci, 1 + tt * P:1 + (tt + 1) * P],
                                     wv[:, ci, :],
                                     start=(ci == 0), stop=(ci == CT - 1))
                nc.vector.tensor_copy(VTx[:, tt, :, 0:HD],
                                      ps.rearrange("p (h e) -> p h e", h=H))

            # ---------- Q/K convs (per output channel tile) + attention ----------
            def conv_qk(dst, w, bias_t, ct):
                for sc in range(2):
                    ps = psp.tile([P, SH], f32, tag="ps", bufs=6, name="psqk")
                    first = True
                    for ci in range(CT):
                        for k in range(KS):
                            nc.tensor.matmul(
                                ps[:], w[:, ci, k, ct * P:(ct + 1) * P],
                                Xb[:, ci, sc * SH + k: sc * SH + k + SH],
                                start=first, stop=(ci == CT - 1 and k == KS - 1))
                            first = False
                    nc.vector.tensor_scalar_add(dst[:, ct, sc * SH:(sc + 1) * SH],
                                                ps[:], bias_t[:, ct:ct + 1])

            def attention_head(h):
                base = HD * (h % 2)
                qh = Q[base:base + HD, h // 2, :]
                kh = K[base:base + HD, h // 2, :]
                for sc in range(2):
                    et = etp.tile([P, ST, SH], bf16, tag="ET", name=f"et{h}_{sc}")
                    for tt in range(ST):
                        ps = psp.tile([P, SH], f32, tag="ps", bufs=6, name="pssc")
                        nc.tensor.matmul(ps[:], kh[:, tt * P:(tt + 1) * P],
                                         qh[:, sc * SH:(sc + 1) * SH],
                                         start=True, stop=True)
                        nc.scalar.activation(et[:, tt, :], ps[:], AF.Exp,
                                             bias=0.0, scale=1.0 / HD)
                    if dbg and h == 0 and sc == 0:
                        nc.sync.dma_start(dbg_d["dET"][:], et[:])
                    # AV: av^T[d, s] accumulated over t tiles; row HD = denom
                    avps = psp.tile([P, SH], f32, tag="ps", bufs=6, name="avps")
                    for tt in range(ST):
                        nc.tensor.matmul(avps[0:HD + 1, :], VTx[:, tt, h, :],
                                         et[:, tt, :],
                                         start=(tt == 0), stop=(tt == ST - 1))
                    # softmax normalization: recip of denom row, broadcast, mult
                    rrec = nrm.tile([1, SH], f32, tag="rrec", name="rrec")
                    nc.vector.reciprocal(rrec[0:1, :], avps[HD:HD + 1, :])
                    rrep = nrm.tile([HD, SH], f32, tag="rrep", name="rrep")
                    nc.gpsimd.partition_broadcast(rrep[:], rrec[0:1, :])
                    base_o = HD * (h % 2)
                    nc.vector.tensor_tensor(
                        out=AVT[base_o:base_o + HD, h // 2, sc * SH:(sc + 1) * SH],
                        in0=avps[0:HD, :], in1=rrep[:], op=ALU.mult)

            for ct in range(CT):
                conv_qk(Q, wq, bq_t, ct)
                conv_qk(K, wk, bk_t, ct)
                attention_head(2 * ct)
                attention_head(2 * ct + 1)

            if dbg:
                nc.sync.dma_start(dbg_d["dQ"][:], Q[:])
                nc.sync.dma_start(dbg_d["dK"][:], K[:])
                nc.sync.dma_start(dbg_d["dV"][:], VTx[:])
                nc.sync.dma_start(dbg_d["dAVT"][:], AVT[:])

            # FFN weights / later tensors into slots freed by earlier phases
            wo = big.tile([P, CT, D], bf16, tag="WC", name="wo_s")
            nc.sync.dma_start(wo[:], wo_d[:])
            w1 = big.tile([P, CT, DFF], bf16, tag="WA", name="w1_s")
            nc.scalar.dma_start(w1[:], w1_d[:])
            w2 = big.tile([P, FT, D], bf16, tag="WB", name="w2_s")
            nc.sync.dma_start(w2[:], w2_d[:])
            # src in [s, d] layout for the residual: PE-transpose X on-chip
            # (saves shipping a second 2MB copy of src per call)
            srcs = big.tile([P, ST, D], f32, tag="V", name="srcs")
            for st in range(ST):
                for ctt in range(CT):
                    tp = psp.tile([P, P], f32, tag="tp", bufs=2, name="tps")
                    nc.tensor.transpose(
                        tp[:], X[:, ctt, 1 + st * P:1 + (st + 1) * P],
                        identity[:])
                    nc.vector.tensor_copy(srcs[:, st, ctt * P:(ctt + 1) * P],
                                          tp[:])

            xs = big.tile([P, ST, D], f32, tag="K", name="xs")  # LN1 out [s, d]
            xT = big.tile([P, CT, S], bf16, tag="Q", name="xT")
            y = big.tile([P, ST, D], f32, tag="A", name="y")

            def layernorm(dst, z, g_t, be_t):  # uses epsv from enclosing scope
                stats = tiny.tile([P, 6], f32, tag="st6", name="st6")
                nc.vector.bn_stats(stats[:], z[:])
                mv = tiny.tile([P, 2], f32, tag="mv", name="mv")
                nc.vector.bn_aggr(mv[:], stats[:])
                sd = tiny.tile([P, 1], f32, tag="sd", name="sd")
                nc.scalar.activation(sd[:], mv[:, 1:2], AF.Sqrt, bias=epsv[:], scale=1.0)
                rstd = tiny.tile([P, 1], f32, tag="rstd", name="rstd")
                nc.vector.reciprocal(rstd[:], sd[:])
                xn = tmp.tile([P, D], f32, tag="t2", bufs=1, name="xn")
                nc.vector.tensor_scalar(out=xn[:], in0=z[:], scalar1=mv[:, 0:1],
                                        scalar2=rstd[:], op0=ALU.subtract,
                                        op1=ALU.mult)
                nc.vector.tensor_tensor(out=xn[:], in0=xn[:], in1=g_t[:],
                                        op=ALU.mult)
                nc.vector.tensor_tensor(out=dst, in0=xn[:], in1=be_t[:],
                                        op=ALU.add)

            # ---------- Wo projection + residual + LN1 ----------
            for st in range(ST):
                ps = psp.tile([P, SH], f32, tag="ps", bufs=6, name="pswo")
                for dt in range(CT):
                    nc.tensor.matmul(ps[:], AVT[:, dt, st * P:(st + 1) * P],
                                     wo[:, dt, :], start=(dt == 0), stop=False)
                nc.tensor.matmul(ps[:], onesrow[:], bo2_t[:],
                                 start=False, stop=True)
                z = tmp.tile([P, D], f32, tag="t1", bufs=1, name="z1")
                nc.vector.tensor_tensor(out=z[:], in0=ps[:], in1=srcs[:, st, :],
                                        op=ALU.add)
                layernorm(xs[:, st, :], z, g1_t, be1_t)

            # ---------- x -> x^T for FFN ----------
            for dt in range(CT):
                for st in range(ST):
                    tp = psp.tile([P, P], f32, tag="tp", bufs=2, name="tp")
                    nc.tensor.transpose(tp[:], xs[:, st, dt * P:(dt + 1) * P],
                                        identity[:])
                    nc.vector.tensor_copy(xT[:, dt, st * P:(st + 1) * P], tp[:])

            if dbg:
                nc.sync.dma_start(dbg_d["dXS"][:], xs[:])
                nc.sync.dma_start(dbg_d["dXT"][:], xT[:])

            # ---------- FFN in two sequence-half passes ----------
            for sc in range(2):
                hT = [etp.tile([P, FT // 2, SH], bf16, tag="ET",
                               name=f"hT{sc}_{i}") for i in range(2)]
                for ft in range(FT):
                    ps = psp.tile([P, SH], f32, tag="ps", bufs=6, name="psf1")
                    for dt in range(CT):
                        nc.tensor.matmul(ps[:], w1[:, dt, ft * P:(ft + 1) * P],
                                         xT[:, dt, sc * SH:(sc + 1) * SH],
                                         start=(dt == 0), stop=(dt == CT - 1))
                    nc.scalar.activation(hT[ft // 8][:, ft % 8, :], ps[:], AF.Relu,
                                         bias=b1_t[:, ft:ft + 1], scale=1.0)
                for j in range(ST // 2):
                    st = sc * (ST // 2) + j
                    ps = psp.tile([P, SH], f32, tag="ps", bufs=6, name="psf2")
                    for ft in range(FT):
                        nc.tensor.matmul(
                            ps[:], hT[ft // 8][:, ft % 8, j * P:(j + 1) * P],
                            w2[:, ft, :], start=(ft == 0), stop=False)
                    nc.tensor.matmul(ps[:], onesrow[:], b2_t[:],
                                     start=False, stop=True)
                    z = tmp.tile([P, D], f32, tag="t1", bufs=1, name="z2")
                    nc.vector.tensor_tensor(out=z[:], in0=ps[:], in1=xs[:, st, :],
                                            op=ALU.add)
                    layernorm(y[:, st, :], z, g2_t, be2_t)
                    nc.sync.dma_start(out_d[:, st, :], y[:, st, :])

      except _PhaseDone:
        pass
    nc.compile()
    return nc


def _prep_inputs(src, Wq, bq, Wk, bk, Wv, bv, Wo, bo, W1, b1, W2, b2,
                 g1, be1, g2, be2):
    import ml_dtypes

    f = np.float32
    bf = ml_dtypes.bfloat16

    def ctile(w):  # [co, ci(, k)] conv weight -> [p, ci_t(, k), co]
        wt = np.ascontiguousarray(np.moveaxis(w, 0, -1))  # [ci(,k), co]
        return np.ascontiguousarray(
            wt.reshape(CT, P, *wt.shape[1:]).transpose(1, 0, *range(2, wt.ndim + 1)))

    shared = {
        "wq": ctile(Wq).astype(bf),                      # [P, CT, KS, D]
        "wk": ctile(Wk).astype(bf),
        "wv": ctile(Wv[:, :, 0]).astype(bf),             # [P, CT, D]
        "wo": np.ascontiguousarray(
            Wo.T.reshape(CT, P, D).transpose(1, 0, 2)).astype(bf),
        "w1": np.ascontiguousarray(
            W1.T.reshape(CT, P, DFF).transpose(1, 0, 2)).astype(bf),
        "w2": np.ascontiguousarray(
            W2.T.reshape(FT, P, D).transpose(1, 0, 2)).astype(bf),
        "bq": np.ascontiguousarray(bq.reshape(CT, P).T).astype(f),
        "bk": np.ascontiguousarray(bk.reshape(CT, P).T).astype(f),
        "b1": np.ascontiguousarray(b1.reshape(FT, P).T).astype(f),
        "bo2": (bo + Wo @ bv).reshape(1, D).astype(bf),
        "b2r": b2.reshape(1, D).astype(bf),
        "g1r": np.asarray(g1).reshape(1, D).astype(f),
        "be1r": np.asarray(be1).reshape(1, D).astype(f),
        "g2r": np.asarray(g2).reshape(1, D).astype(f),
        "be2r": np.asarray(be2).reshape(1, D).astype(f),
        "ident": np.eye(P, dtype=f),
        "onesrow": np.ones((1, P), dtype=bf),
    }
    in_maps = []
    for b in range(NCORES):
        m = dict(shared)
        m["srcT"] = np.ascontiguousarray(
            src[b].T.reshape(CT, P, S).transpose(1, 0, 2)).astype(f)
        in_maps.append(m)
    return in_maps


def get_nc():
    if "nc" not in _CACHE:
        _CACHE["nc"] = _build_nc()
    return _CACHE["nc"]


def kernel(**inputs):
    from concourse.bass_utils import run_bass_kernel_spmd

    in_maps = _prep_inputs(**{k: np.asarray(v) for k, v in inputs.items()})
    nc = get_nc()
    res = run_bass_kernel_spmd(nc, in_maps, core_ids=list(range(NCORES)))
    outs = [r["out"].transpose(1, 0, 2).reshape(S, D) for r in res.results]
    return np.stack(outs).astype(np.float32)

